# revision 12
# baseline (speedup 1.0000x reference)
"""Trainium2 Bass kernel for nn_EnhancedCell (data-parallel over batch, 8 cores).

kernel(**inputs) takes FULL unsharded inputs (feats [64,512,768], lengths [64],
labelss [64,512], plus weights) and returns the scalar loss matching
reference.reference().

Per-core plan (8 batch rows/core):
  - feats cast fp32->bf16 during the (single, per-row) SWDGE DMA; PE transposes
    build FT [768(d parts), 516(pos free)] in bf16; the length mask and pad_end
    replacement are applied exactly during eviction:
    ft = (psum - pad_end) * mask_bc + pad_end.
  - All matmuls in bf16 (weights pre-cast on host into packed inline tensors);
    accumulation is fp32 in PSUM. L1 = 5 shifted D->H projections on shifted FT
    windows; embedding path host-folded into a [68,256] one-hot table; gates +
    attention contract cat^T chunks.
  - Gates via tanh (sigmoid(x)=0.5*tanh(x/2)+0.5) so the whole kernel stays in
    one ACT table set (exp_and_others: exp/tanh/relu/copy).
  - Hidden combine pushed through per-gate output matmuls; attention weights
    (incl. softmax normalizer) applied as per-token scalars in [token, TAG]
    layout, batched across the 4 token tiles. Final log-softmax stats in fp32.
  - Device emits per-token (logit[label]-max) and sum(exp(logit-max)); host
    finishes loss = -sum(mask*max(g-ln(se), ln 1e-9))/max(sum(mask),1).
"""

import sys
import numpy as np

if "/opt/trn_rl_repo" not in sys.path:
    sys.path.insert(0, "/opt/trn_rl_repo")

B, S, D, H, TAG, E, PP, NP, NN = 64, 512, 768, 256, 32, 64, 2, 2, 2
NC = 8
BC = B // NC            # batch rows per core
KD = D // 128           # 6 d-chunks
MC = H // 128           # 2 h-chunks
TT = S // 128           # 4 token tiles
LPOS = NP + S + NN      # 516 padded positions
LOG_EPS = float(np.log(1e-9))

_CACHE = {}
LAST_RESULTS = None

# layout of the packed bf16 const block [128, BF_COLS]
_BF_SEGS = [("WT5", KD * 5 * MC * 128), ("WzT", 4 * 4 * MC * 128),
            ("WaT", 4 * 4), ("WoT", MC * TAG), ("ident", 128)]
# layout of the packed fp32 const block [128, F32_COLS]
_F32_SEGS = [("bL1", 8), ("bzh", 8), ("bout", TT * TAG), ("padbng", KD),
             ("padend", KD), ("iotatag", TT * TAG)]


def _seg_off(segs, name):
    off = 0
    for n, w in segs:
        if n == name:
            return off, w
        off += w
    raise KeyError(name)


def prep_weights(inp):
    import ml_dtypes
    bf = ml_dtypes.bfloat16
    f = lambda k: np.asarray(inp[k], dtype=np.float32)
    W_hp, W_hc, W_hn = f("W_hp"), f("W_hc"), f("W_hn")
    W_pe, emb = f("W_pe"), f("emb_table")

    parts = np.stack([W_hp[:, :D], W_hp[:, D:], W_hc, W_hn[:, :D], W_hn[:, D:]])
    WT5 = parts.reshape(5, MC, 128, KD, 128).transpose(4, 3, 0, 1, 2)  # [128,kc,s,mc,m]

    Wz = np.stack([f("Wz_pe"), f("Wz_hp"), f("Wz_hc"), f("Wz_hn")])  # [4,256,512]
    WzT = Wz.reshape(4, MC, 128, 4, 128).transpose(4, 3, 0, 1, 2)  # [128,kc,g,mc,m]

    WaT = f("W_att").reshape(4, 4, 128).transpose(2, 1, 0)  # [128, kc, 4]
    WoT = f("W_out").T.reshape(MC, 128, TAG).transpose(1, 0, 2)  # [128, mc, 32]

    bigbf = np.concatenate([WT5.reshape(128, -1), WzT.reshape(128, -1),
                            WaT.reshape(128, -1), WoT.reshape(128, -1),
                            np.eye(128, dtype=np.float32)], axis=1).astype(bf)

    def col2(v):
        return np.asarray(v, np.float32).reshape(MC, 128).T
    bL1 = np.concatenate([col2(inp["b_hp"]), col2(inp["b_hc"]),
                          col2(inp["b_hn"]), col2(inp["b_pe"])], axis=1)
    bzh = 0.5 * np.concatenate([col2(inp["bz_pe"]), col2(inp["bz_hp"]),
                                col2(inp["bz_hc"]), col2(inp["bz_hn"])], axis=1)
    pad_bng = np.asarray(inp["pad_bng"], np.float32).reshape(D)
    pad_end = np.asarray(inp["pad_end"], np.float32).reshape(D)

    bigf32 = np.concatenate([
        bL1, bzh,
        np.tile(np.asarray(inp["b_out"], np.float32)[None, :], (128, TT)),
        pad_bng.reshape(KD, 128).T, pad_end.reshape(KD, 128).T,
        np.tile(np.arange(TAG, dtype=np.float32)[None, :], (128, TT)),
    ], axis=1).astype(np.float32)

    # [68, 261] bf16: MT2 (0:256), ident4 rows 0-3 cols 256:260, iota34 col 260
    MT2 = np.concatenate([emb @ W_pe[:, :E].T, emb @ W_pe[:, E:].T], axis=0)
    m68 = np.zeros((68, 260), np.float32)
    m68[:, :256] = MT2
    m68[0:4, 256:260] = np.eye(4, dtype=np.float32)
    aux68 = np.zeros((68, 2), np.float32)
    aux68[:, 0] = np.arange(68) % 34
    aux68[0:4, 1] = np.asarray(inp["b_att"], np.float32).reshape(4)
    return {
        "bigbf": bigbf,
        "bigf32": bigf32,
        "m68bf": m68.astype(bf),
        "aux68": aux68,
    }


def build_bass(consts, bc=BC):
    import concourse.bacc as bacc
    import concourse.tile as tile
    import concourse.bass as bass
    from concourse import mybir
    from contextlib import ExitStack

    f32 = mybir.dt.float32
    bf16 = mybir.dt.bfloat16
    Alu = mybir.AluOpType
    Act = mybir.ActivationFunctionType
    AX = mybir.AxisListType.X

    nc = bacc.Bacc("TRN2", target_bir_lowering=False, debug=False,
                   enable_asserts=True, num_devices=NC, enable_partition_id=False)

    feats_t = nc.dram_tensor("feats", [bc, S, D], f32, kind="ExternalInput").ap()
    labext_t = nc.dram_tensor("labext", [bc, 2 + S], f32, kind="ExternalInput").ap()
    msk_t = nc.dram_tensor("msk", [bc, S], f32, kind="ExternalInput").ap()
    out_t = nc.dram_tensor("out", [bc, 128, 2 * TT], f32, kind="ExternalOutput").ap()

    cdram = {k: nc.inline_tensor(np.ascontiguousarray(v), k).ap()
             for k, v in consts.items()}

    def bcast_ap(src_ap, nparts):
        return bass.AP(tensor=src_ap.tensor, offset=src_ap.offset,
                       ap=[[0, nparts]] + [list(p) for p in src_ap.ap])

    names = ["pe", "hp", "hc", "hn"]

    with tile.TileContext(nc) as tc:
        with ExitStack() as ctx:
            const = ctx.enter_context(tc.tile_pool(name="const", bufs=1))
            ftp = ctx.enter_context(tc.tile_pool(name="ftp", bufs=1))
            fnp = ctx.enter_context(tc.tile_pool(name="fnp", bufs=2))
            hp_ = ctx.enter_context(tc.tile_pool(name="hp_", bufs=1))
            zp_ = ctx.enter_context(tc.tile_pool(name="zp_", bufs=1))
            gp_ = ctx.enter_context(tc.tile_pool(name="gp_", bufs=1))
            ohp = ctx.enter_context(tc.tile_pool(name="ohp", bufs=2))
            smp = ctx.enter_context(tc.tile_pool(name="smp", bufs=2))
            fip = ctx.enter_context(tc.tile_pool(name="fip", bufs=2))
            dmp = ctx.enter_context(tc.tile_pool(name="dmp", bufs=2))
            pbig = ctx.enter_context(tc.tile_pool(name="pbig", bufs=4, space="PSUM"))
            psml = ctx.enter_context(tc.tile_pool(name="psml", bufs=4, space="PSUM"))

            cbf = const.tile([128, sum(w for _, w in _BF_SEGS)], bf16,
                             name="cbf", tag="cbf")
            nc.sync.dma_start(out=cbf[...], in_=cdram["bigbf"][...])
            cf32 = const.tile([128, sum(w for _, w in _F32_SEGS)], f32,
                              name="cf32", tag="cf32")
            nc.sync.dma_start(out=cf32[...], in_=cdram["bigf32"][...])
            c68 = const.tile([68, 260], bf16, name="c68", tag="c68")
            nc.sync.dma_start(out=c68[...], in_=cdram["m68bf"][...])
            aux68 = const.tile([68, 2], f32, name="aux68", tag="aux68")
            nc.sync.dma_start(out=aux68[...], in_=cdram["aux68"][...])
            cones = const.tile([1, 128], bf16, name="cones", tag="cones")
            nc.vector.memset(cones[...], 1.0)

            def bfseg(name):
                off, w = _seg_off(_BF_SEGS, name)
                return cbf[:, off:off + w]

            def f32seg(name):
                off, w = _seg_off(_F32_SEGS, name)
                return cf32[:, off:off + w]

            WT5v = bfseg("WT5").rearrange("p (kc s mc m) -> p kc s mc m",
                                          kc=KD, s=5, mc=MC)
            WzTv = bfseg("WzT").rearrange("p (kc g mc m) -> p kc g mc m",
                                          kc=4, g=4, mc=MC)
            WaTv = bfseg("WaT").rearrange("p (kc j) -> p kc j", kc=4)
            WoTv = bfseg("WoT").rearrange("p (mc o) -> p mc o", mc=MC)
            identv = bfseg("ident")
            MT2v = c68[:, 0:256]
            ident4v = c68[0:4, 256:260]
            iota34v = aux68[:, 0:1]
            cbatt = aux68

            # persistent FT (bf16) [128, KD*LPOS]; fixed pad cols written once
            ft = ftp.tile([128, KD * LPOS], bf16, name="ft", tag="ft")
            for kc in range(KD):
                base = kc * LPOS
                for c in (0, 1):
                    nc.vector.tensor_copy(ft[:, base + c: base + c + 1],
                                          f32seg("padbng")[:, kc:kc + 1])
                for c in (NP + S, NP + S + 1):
                    nc.vector.tensor_copy(ft[:, base + c: base + c + 1],
                                          f32seg("padend")[:, kc:kc + 1])

            for b in range(bc):
                mskrow = smp.tile([1, S], bf16, name="mskrow", tag="mskrow")
                nc.gpsimd.dma_start(out=mskrow[...], in_=msk_t[b:b + 1, :])
                lab4 = smp.tile([128, TT], f32, name="lab4", tag="lab4")
                nc.sync.dma_start(out=lab4[...],
                                  in_=labext_t[b, 2:2 + S].rearrange("(t p) -> p t", p=128))
                labbc = ohp.tile([68, S], f32, name="labbc", tag="labbc")
                nc.sync.dma_start(out=labbc[0:34, :], in_=bcast_ap(labext_t[b, 0:S], 34))
                nc.sync.dma_start(out=labbc[34:68, :], in_=bcast_ap(labext_t[b, 1:1 + S], 34))
                oh2 = ohp.tile([68, S], bf16, name="oh2", tag="oh2")
                nc.vector.tensor_scalar(out=oh2[...], in0=labbc[...],
                                        scalar1=iota34v, scalar2=None,
                                        op0=Alu.is_equal)

                # mask broadcast [128, S] via PE rank-1
                pmb = pbig.tile([128, S], f32, name="pmb", tag="pbig")
                nc.tensor.matmul(pmb[...], lhsT=cones[...], rhs=mskrow[...],
                                 start=True, stop=True)
                mbc = dmp.tile([128, S], f32, name="mbc", tag="mbc")
                nc.scalar.copy(mbc[...], pmb[...])

                # single cast-DMA of the whole row's features (fp32 -> bf16)
                fnat = fnp.tile([128, TT, D], bf16, name="fnat", tag="fnat")
                nc.gpsimd.dma_start(
                    out=fnat[...],
                    in_=feats_t[b].rearrange("(tt p) d -> p tt d", p=128))

                # ---- transposes into FT; eviction = (pt - pad) * m + pad
                for kc in range(KD):
                    ptk = psml.tile([128, S], bf16, name="ptk", tag="psml")
                    for tt in range(TT):
                        nc.tensor.matmul(ptk[:, tt * 128:(tt + 1) * 128],
                                         lhsT=fnat[:, tt, kc * 128:(kc + 1) * 128],
                                         rhs=identv, is_transpose=True,
                                         start=True, stop=True)
                    dst = ft[:, kc * LPOS + NP: kc * LPOS + NP + S]
                    nc.vector.scalar_tensor_tensor(
                        out=dst, in0=ptk[...], scalar=f32seg("padend")[:, kc:kc + 1],
                        in1=mbc[...], op0=Alu.subtract, op1=Alu.mult)
                    nc.vector.tensor_scalar_add(dst, dst, f32seg("padend")[:, kc:kc + 1])

                # ---- L1 projections (h^T layout), outputs bf16
                hT = {}
                for mc in range(MC):
                    for x, slist in (("hp", (0, 1)), ("hc", (2,)), ("hn", (3, 4))):
                        ps = pbig.tile([128, S], f32, name="psl1", tag="pbig")
                        mms = [(s, kc) for s in slist for kc in range(KD)]
                        for i, (s, kc) in enumerate(mms):
                            nc.tensor.matmul(
                                ps[...], lhsT=WT5v[:, kc, s, mc, :],
                                rhs=ft[:, kc * LPOS + s: kc * LPOS + s + S],
                                start=(i == 0), stop=(i == len(mms) - 1))
                        h = hp_.tile([128, S], bf16, name=f"h_{x}{mc}", tag=f"h_{x}{mc}")
                        bcol = {"hp": 0, "hc": 1, "hn": 2}[x] * 2 + mc
                        nc.scalar.activation(h[...], ps[...], Act.Relu,
                                             bias=f32seg("bL1")[:, bcol:bcol + 1])
                        hT[(x, mc)] = h
                    ps = pbig.tile([128, S], f32, name="pspe", tag="pbig")
                    nc.tensor.matmul(ps[...], lhsT=MT2v[:, mc * 128:(mc + 1) * 128],
                                     rhs=oh2[...], start=True, stop=True)
                    h = hp_.tile([128, S], bf16, name=f"h_pe{mc}", tag=f"h_pe{mc}")
                    nc.scalar.activation(h[...], ps[...], Act.Relu,
                                         bias=f32seg("bL1")[:, 6 + mc:7 + mc])
                    hT[("pe", mc)] = h

                catT = [hT[("pe", 0)], hT[("pe", 1)], hT[("hc", 0)], hT[("hc", 1)]]

                # ---- gates via tanh; g_x = (t+1)*h_x  (bf16)
                gx = {}
                for g in range(4):
                    for mc in range(MC):
                        ps = pbig.tile([128, S], f32, name="psg", tag="pbig")
                        for kc in range(4):
                            nc.tensor.matmul(ps[...], lhsT=WzTv[:, kc, g, mc, :],
                                             rhs=catT[kc][...],
                                             start=(kc == 0), stop=(kc == 3))
                        t = zp_.tile([128, S], bf16, name=f"t_{g}{mc}", tag=f"t_{g}{mc}")
                        bcol = g * 2 + mc
                        nc.scalar.activation(t[...], ps[...], Act.Tanh, scale=0.5,
                                             bias=f32seg("bzh")[:, bcol:bcol + 1])
                        u = gp_.tile([128, S], bf16, name=f"g_{g}{mc}", tag=f"g_{g}{mc}")
                        nc.vector.scalar_tensor_tensor(out=u[...], in0=t[...], scalar=1.0,
                                                       in1=hT[(names[g], mc)][...],
                                                       op0=Alu.add, op1=Alu.mult)
                        gx[(names[g], mc)] = u

                # ---- attention exp weights e4 [4, S] bf16 (unnormalized)
                pa = psml.tile([4, S], f32, name="pa", tag="psml")
                for kc in range(4):
                    nc.tensor.matmul(pa[...], lhsT=WaTv[:, kc, :], rhs=catT[kc][...],
                                     start=(kc == 0), stop=(kc == 3))
                e4 = smp.tile([4, S], bf16, name="e4", tag="e4")
                nc.scalar.activation(e4[...], pa[...], Act.Exp, bias=cbatt[0:4, 1:2])

                # ---- batched final phase over all 4 token tiles
                pet = psml.tile([128, TT * 4], bf16, name="pet", tag="psml")
                for tt in range(TT):
                    nc.tensor.matmul(pet[:, tt * 4:(tt + 1) * 4],
                                     lhsT=e4[:, tt * 128:(tt + 1) * 128],
                                     rhs=ident4v, is_transpose=True,
                                     start=True, stop=True)
                eT = fip.tile([128, TT * 4], f32, name="eT", tag="eT")
                nc.vector.tensor_copy(eT[...], pet[...])
                sT4 = fip.tile([128, TT], f32, name="sT4", tag="sT4")
                nc.vector.tensor_reduce(out=sT4[...],
                                        in_=eT[...].rearrange("p (t x) -> p t x", x=4),
                                        axis=AX, op=Alu.add)
                rT4 = fip.tile([128, TT], f32, name="rT4", tag="rT4")
                nc.vector.reciprocal(rT4[...], sT4[...])
                att4 = fip.tile([128, TT * 4], f32, name="att4", tag="att4")
                rbc = bass.AP(tensor=rT4[...].tensor, offset=rT4[...].offset,
                              ap=[list(rT4[...].ap[0]), list(rT4[...].ap[1]), [0, 4]])
                nc.vector.scalar_tensor_tensor(
                    out=att4[...].rearrange("p (t x) -> p t x", x=4),
                    in0=eT[...].rearrange("p (t x) -> p t x", x=4),
                    scalar=0.5, in1=rbc, op0=Alu.mult, op1=Alu.mult)

                # per-gate logits for all tiles: plo [128, (tt, g, o)]
                plo = psml.tile([128, TT * 4 * TAG], f32, name="plo", tag="psml")
                for tt in range(TT):
                    for g in range(4):
                        for mc in range(MC):
                            nc.tensor.matmul(
                                plo[:, (tt * 4 + g) * TAG:(tt * 4 + g + 1) * TAG],
                                lhsT=gx[(names[g], mc)][:, tt * 128:(tt + 1) * 128],
                                rhs=WoTv[:, mc, :],
                                start=(mc == 0), stop=(mc == MC - 1))

                plo4 = plo[...].rearrange("p (t g o) -> p t g o", g=4, o=TAG)
                att43 = att4[...].rearrange("p (t x) -> p t x", x=4)

                def attbc(g):
                    a = att43[:, :, g:g + 1]
                    return bass.AP(tensor=a.tensor, offset=a.offset,
                                   ap=[list(a.ap[0]), list(a.ap[1]), [0, TAG]])

                lsc = fip.tile([128, TT, TAG], f32, name="lsc", tag="lsc")
                tmp = fip.tile([128, TT, TAG], f32, name="tmp", tag="tmp")
                nc.vector.tensor_tensor(out=lsc[...], in0=plo4[:, :, 0, :],
                                        in1=attbc(0), op=Alu.mult)
                for g in range(1, 4):
                    nc.vector.tensor_tensor(out=tmp[...], in0=plo4[:, :, g, :],
                                            in1=attbc(g), op=Alu.mult)
                    nc.vector.tensor_tensor(out=lsc[...], in0=lsc[...], in1=tmp[...],
                                            op=Alu.add)
                nc.vector.tensor_tensor(
                    out=lsc[...], in0=lsc[...],
                    in1=f32seg("bout")[...].rearrange("p (t o) -> p t o", o=TAG),
                    op=Alu.add)

                gs = fip.tile([128, 2 * TT], f32, name="gs", tag="gs")
                negmx = fip.tile([128, TT], f32, name="negmx", tag="negmx")
                nc.vector.tensor_reduce(out=negmx[...], in_=lsc[...], axis=AX,
                                        op=Alu.max, negate=True)

                def bc4(t2d):
                    a = t2d[...]
                    return bass.AP(tensor=a.tensor, offset=a.offset,
                                   ap=[list(a.ap[0]), list(a.ap[1]), [0, TAG]])

                xs = fip.tile([128, TT, TAG], f32, name="xs", tag="xs")
                nc.gpsimd.tensor_tensor(out=xs[...], in0=lsc[...], in1=bc4(negmx),
                                        op=Alu.add)
                es = fip.tile([128, TT, TAG], f32, name="es", tag="es")
                nc.scalar.activation(es[...], xs[...], Act.Exp)
                nc.vector.tensor_reduce(out=gs[:, TT:2 * TT], in_=es[...], axis=AX,
                                        op=Alu.add)
                ohh = fip.tile([128, TT, TAG], f32, name="ohh", tag="ohh")
                nc.vector.tensor_tensor(
                    out=ohh[...],
                    in0=f32seg("iotatag")[...].rearrange("p (t o) -> p t o", o=TAG),
                    in1=bc4(lab4), op=Alu.is_equal)
                gl = fip.tile([128, TT, TAG], f32, name="gl", tag="gl")
                nc.gpsimd.tensor_tensor(out=gl[...], in0=ohh[...], in1=xs[...],
                                        op=Alu.mult)
                nc.vector.tensor_reduce(out=gs[:, 0:TT], in_=gl[...], axis=AX,
                                        op=Alu.add)
                nc.sync.dma_start(out=out_t[b], in_=gs[...])

    nc.compile()
    return nc


def finish_loss(outs, labels):
    """outs: [ncores, bc, 128, 2*TT]; labels: [ncores*bc, S] ints."""
    ncores, bc = outs.shape[0], outs.shape[1]
    glx = outs[:, :, :, 0:TT].transpose(0, 1, 3, 2).reshape(ncores * bc, S)
    se = outs[:, :, :, TT:2 * TT].transpose(0, 1, 3, 2).reshape(ncores * bc, S)
    mask = (labels != -1)
    logp = np.maximum(glx.astype(np.float64) - np.log(se.astype(np.float64)), LOG_EPS)
    total = float((logp * mask).sum())
    count = max(int(mask.sum()), 1)
    return np.float32(-total / count)


def make_in_maps(inputs):
    feats = np.ascontiguousarray(np.asarray(inputs["feats"], dtype=np.float32))
    lengths = np.asarray(inputs["lengths"]).astype(np.int64)
    labels = np.asarray(inputs["labelss"]).astype(np.int64)

    labext = np.zeros((B, 2 + S), np.float32)
    labext[:, 0] = TAG
    labext[:, 1] = TAG + 1
    labext[:, 2:] = labels.astype(np.float32)
    msk = (np.arange(S)[None, :] < lengths[:, None]).astype(np.float32)

    in_maps = []
    for c in range(NC):
        sl = slice(c * BC, (c + 1) * BC)
        in_maps.append({
            "feats": feats[sl],
            "labext": np.ascontiguousarray(labext[sl]),
            "msk": np.ascontiguousarray(msk[sl]),
        })
    return in_maps, labels


def kernel(**inputs):
    global LAST_RESULTS
    from concourse.bass_utils import run_bass_kernel_spmd

    consts = prep_weights(inputs)
    if "nc" not in _CACHE:
        _CACHE["nc"] = build_bass(consts)
    nc = _CACHE["nc"]

    in_maps, labels = make_in_maps(inputs)
    res = run_bass_kernel_spmd(nc, in_maps, core_ids=list(range(NC)))
    LAST_RESULTS = res

    outs = np.stack([res.results[c]["out"] for c in range(NC)])
    return finish_loss(outs, labels)
